# revision 5
# baseline (speedup 1.0000x reference)
"""Exponential Hawkes process negative log-likelihood on 8 Trainium2 cores.

Math (reference):
    R_0 = 0;  R_i = exp(-beta*(t_i - t_{i-1})) * (1 + R_{i-1})
    lam_i = mu + alpha * R_i
    nll = -[ sum_i log(lam_i) - mu*T - (alpha/beta) * sum_i (1 - exp(-beta*(T - t_i)))
             - 1000 * relu(alpha/beta - 0.999)^2 ]

Strategy (blocked scan, per the sharding hint):
  - The kernel's device input is dt_i = t_i - t_{i-1} (the same event data,
    differenced on the host during sharding and shipped as bf16 -- half the
    HBM traffic of f32 t, and it removes the shifted-subtract from the
    device's Vector engine, whose serial scan is the bottleneck).
  - Shard the 8.4M event axis across 8 cores, each shard prefixed with a
    1536-event halo so the incoming recurrence carry is reproduced locally
    (exp(-beta * halo_span) underflows to 0 in f32; verified host-side).
    Core 0 is front-padded with dt = 1e6, whose a = exp(-beta dt) = 0
    resets the recurrence exactly.
  - Per core the (halo+shard) dt sequence is laid out [128, C]: partition p
    owns a contiguous chunk of C events.  Per column tile:
      a    = exp(-beta*dt)                 (ACT, bf16 in / f32 out)
      B    = scan: B_c = a_c*(1+B_{c-1})   (DVE tensor_tensor_scan,
                                            ~2 cyc/elem, chained via the
                                            previous tile's last column)
      lnl  = Ln(alpha*B + mu)              (ACT, bf16 out, DMA'd back)
  - The host reduces the shipped per-event bf16 log terms in f64, masking
    the halo entries and the first W_c columns of each partition chunk,
    whose cross-partition carry correction it recomputes exactly (f64)
    from B_head/B_end -- the correction factor decays to exactly 0 (f32)
    past W_c, which is verified against the data host-side.
  - The integral sum_i exp(-beta*(T - t_i)) is computed entirely on the
    host in f64: only events within ~700/beta of T contribute above
    1e-300, a tiny tail (for the reference data ~300 events).
"""

import numpy as np
import ml_dtypes

# Problem constants (hardcoded per task instructions).
N = 8_388_608          # total events
M = 8                  # cores
S = N // M             # events per shard (1,048,576)
H = 1536               # halo events prepended to each shard; equals the
                       # tile-boundary prefix so the halo split is aligned
L = S + H              # per-core sequence length
P = 128                # SBUF partitions
C = L // P             # columns per partition (8204)
EPS = 1e-8
PENALTY = 1000.0
PAD_GAP = 1.0e6        # core-0 pad dt; exp(-beta*PAD_GAP) == 0 in f32

# Column tiles (start, width).  Small head tiles so the serial scan chain
# starts while the DMA queues are still ramping; small tail tile so the
# last Ln/DMA finish quickly.  Halo boundary (H = 1536) falls after tile 1.
# Fallback config with a 1536-wide head tile for smaller beta (the carry
# window W_c must fit inside tile 0).
_TILES_A = [(0, 512), (512, 1024), (1536, 2952), (4488, 2696), (7184, 1020)]
_TILES_B = [(0, 1536), (1536, 2048), (3584, 2296), (5880, 2324)]
assert sum(w for _, w in _TILES_A) == C and sum(w for _, w in _TILES_B) == C
assert _TILES_A[0][1] + _TILES_A[1][1] == H and _TILES_B[0][1] == H

_PROGRAM_CACHE: dict = {}


def _softplus64(x: float) -> float:
    return float(np.logaddexp(0.0, np.float64(x)))


def _build_program(beta: float, mu: float, alpha: float, tiles: tuple):
    import concourse.bacc as bacc
    import concourse.mybir as mybir
    from concourse.tile import TileContext

    f32 = mybir.dt.float32
    bf16 = mybir.dt.bfloat16
    AF = mybir.ActivationFunctionType
    OP = mybir.AluOpType
    NT = len(tiles)
    W0 = tiles[0][1]
    FMAX = max(w for _, w in tiles)

    # This kernel interleaves Exp and Ln activations per tile.  The stock
    # table chooser picks the first act-func-set containing each function,
    # which alternates between an Exp-only and an Ln-only set and inserts an
    # ACT_TABLE_LOAD (~1.3us) at every switch.  Hide Exp/Ln from all sets
    # except the combined one (order/indices preserved) so both functions
    # resolve to a single resident table.
    if not getattr(bacc, "_hawkes_act_tables_patched", False):
        _orig_get_tables = bacc.get_activation_tables

        def _patched_get_tables(module_arch):
            tabs = _orig_get_tables(module_arch)
            both = {name for name, s in tabs.items()
                    if AF.Exp in s and AF.Ln in s}
            if both:
                keep = next(iter(sorted(both)))
                tabs = {
                    name: (s if name == keep
                           else s - {AF.Exp, AF.Ln})
                    for name, s in tabs.items()
                }
            return tabs

        bacc.get_activation_tables = _patched_get_tables
        bacc._hawkes_act_tables_patched = True

    nc = bacc.Bacc()
    dt_in = nc.dram_tensor("dt", [P, C], bf16, kind="ExternalInput")
    out_lnl = nc.dram_tensor("out_lnl", [P, C], bf16, kind="ExternalOutput")
    out_bhead = nc.dram_tensor("out_bhead", [P, W0], f32,
                               kind="ExternalOutput")
    out_bend = nc.dram_tensor("out_bend", [P, 1], f32, kind="ExternalOutput")

    with TileContext(nc) as tc:
        with tc.tile_pool(name="pers", bufs=1) as pers, \
             tc.tile_pool(name="work", bufs=3) as work:
            musb = pers.tile([P, 1], f32)
            nc.gpsimd.memset(musb[:], float(mu))
            prev_b = None
            for j, (c0, w) in enumerate(tiles):
                dtt = work.tile([P, FMAX], bf16, tag="dt")
                nc.sync.dma_start(dtt[:, :w], dt_in[:, c0:c0 + w])

                at = work.tile([P, FMAX], f32, tag="a")
                nc.scalar.activation(at[:, :w], dtt[:, :w], AF.Exp,
                                     scale=float(-beta))

                bt = work.tile([P, FMAX], f32, tag="b")
                init = 0.0 if j == 0 else prev_b
                nc.vector.tensor_tensor_scan(
                    bt[:, :w], at[:, :w], at[:, :w], init,
                    op0=OP.mult, op1=OP.add)

                if j == 0:
                    # ship the carry-head block as soon as it exists
                    nc.sync.dma_start(out_bhead[:], bt[:, :W0])

                lnl = work.tile([P, FMAX], bf16, tag="lnl")
                nc.scalar.activation(lnl[:, :w], bt[:, :w], AF.Ln,
                                     scale=float(alpha), bias=musb[:])
                nc.sync.dma_start(out_lnl[:, c0:c0 + w], lnl[:, :w])

                if j == NT - 1:
                    nc.sync.dma_start(out_bend[:], bt[:, w - 1:w],
                                      single_packet=True)
                prev_b = bt[:, w - 1:w]

    nc.finalize()
    return nc


def _get_program(beta, mu, alpha, tiles):
    key = (repr(beta), repr(mu), repr(alpha), tuple(tiles))
    prog = _PROGRAM_CACHE.get(key)
    if prog is None:
        prog = _build_program(beta, mu, alpha, tiles)
        _PROGRAM_CACHE[key] = prog
    return prog


def kernel(event_times, raw_mu, raw_alpha, raw_beta, _want_trace=False):
    from concourse.bass_utils import run_bass_kernel_spmd

    ev_full = np.ascontiguousarray(np.asarray(event_times, dtype=np.float32))
    assert ev_full.shape == (N,), ev_full.shape
    mu = _softplus64(float(np.asarray(raw_mu))) + EPS
    alpha = _softplus64(float(np.asarray(raw_alpha))) + EPS
    beta = _softplus64(float(np.asarray(raw_beta))) + EPS
    T = float(ev_full[-1])

    # dt in f32 (same subtraction the reference's recurrence sees), then
    # bf16 for the device; dt[0] has no predecessor -> a must be 0.
    dt_full = np.empty(N, np.float32)
    dt_full[0] = PAD_GAP
    np.subtract(ev_full[1:], ev_full[:-1], out=dt_full[1:])
    dt16_full = dt_full.astype(ml_dtypes.bfloat16)

    # halo sufficiency: the carry truncated at each shard/halo start must
    # have decayed to 0 (in f32) before the first real event.
    halo_span = ev_full[np.arange(1, M) * S] - ev_full[np.arange(1, M) * S - H]
    if not np.all(beta * halo_span.astype(np.float64) > 120.0):
        raise RuntimeError(f"halo H={H} insufficient for beta={beta}")

    # Per-core inputs and host-side fixup metadata
    in_maps = []
    prevs = []     # predecessor event time of each partition chunk
    t2ds = []      # per-core [P, C] event-time windows (for f64 fixups)
    wc_req = 0
    for k in range(M):
        if k == 0:
            win_dt = np.empty(L, ml_dtypes.bfloat16)
            win_dt[:H] = ml_dtypes.bfloat16(PAD_GAP)
            win_dt[H:] = dt16_full[:S]
            win_t = np.empty(L, np.float32)
            win_t[:H] = ev_full[0] - np.float32(PAD_GAP)
            win_t[H:] = ev_full[:S]
        else:
            win_dt = dt16_full[k * S - H:(k + 1) * S]
            win_t = ev_full[k * S - H:(k + 1) * S]
        pv = np.empty(P, np.float64)
        pv[0] = np.float64(win_t[0]) - 1.0
        pv[1:] = win_t[C - 1:L - 1:C]
        t2d = win_t.reshape(P, C)
        # first column where beta*(t - t_chunk0) > 110 (margin over the
        # f32 exp underflow at ~104); beyond it the carry/init correction
        # has decayed to exactly 0 in f32
        past = t2d > (t2d[:, 0:1] + np.float32(110.0 / beta))
        if k == 0:
            past = past[1:]   # pad row: a = 0 resets exactly, never binds
        if not past[:, -1].all():
            wc_req = C        # pathological: no underflow within the row
        else:
            wc_req = max(wc_req, int(past.argmax(axis=1).max()))
        in_maps.append({"dt": np.ascontiguousarray(win_dt.reshape(P, C))})
        prevs.append(pv)
        t2ds.append(t2d)

    tiles = _TILES_A
    w_carry = min(-(-max(wc_req + 64, 128) // 64) * 64, tiles[0][1])
    if wc_req + 16 > w_carry:
        tiles = _TILES_B
        w_carry = min(-(-max(wc_req + 64, 128) // 64) * 64, tiles[0][1])
        if wc_req + 16 > w_carry:
            raise RuntimeError(
                f"carry window {wc_req} exceeds tile width {tiles[0][1]}; "
                f"beta={beta} too small for this build")

    prog = _get_program(beta, mu, alpha, tuple(tiles))
    res = run_bass_kernel_spmd(prog, in_maps, list(range(M)),
                               trace=_want_trace)

    Wc = w_carry
    log_term = np.float64(0.0)
    for k in range(M):
        r = res.results[k]
        lnl = r["out_lnl"].astype(np.float32)           # [P, C]
        # device log terms: all rows past the carry window, plus row 0's
        # real (non-halo) region; the [0, Wc) head is recomputed below.
        log_term += lnl[1:, Wc:].sum(dtype=np.float64)
        log_term += lnl[0, H:].sum(dtype=np.float64)

        # Host-side carry, all f64.  The device scanned each partition chunk
        # with init 0; the true carry-in K[p] adds P_c*K[p] with
        # P_c = prod_{j<=c} a_j ~= exp(-beta*(t_c - prev_p)), which has
        # decayed to exactly 0 (f32) for c >= Wc.
        t2d = t2ds[k].astype(np.float64)
        pv = prevs[k]
        bend = r["out_bend"].astype(np.float64)[:, 0]            # [P]
        Pend = np.exp(-beta * (t2d[:, C - 1] - pv))              # [P]
        K = np.empty(P, np.float64)
        rend = 0.0
        for p in range(P):
            K[p] = rend
            rend = bend[p] + Pend[p] * rend
        bhead = r["out_bhead"].astype(np.float64)[:, :Wc]        # [P, Wc]
        Phead = np.exp(-beta * (t2d[:, :Wc] - pv[:, None]))      # [P, Wc]
        R = bhead + Phead * K[:, None]
        lncorr = np.log(mu + alpha * R)                          # [P, Wc]
        log_term += lncorr[1:, :].sum()                          # row 0: halo

    # Integral term fully on host (f64): only events within ~700/beta of T
    # contribute above 1e-300.
    lo = int(np.searchsorted(ev_full, np.float32(T - 700.0 / beta)))
    int_exp = float(np.exp(-beta * (np.float64(T) -
                                    ev_full[lo:].astype(np.float64))).sum())
    integral_term = mu * T + (alpha / beta) * (N - int_exp)

    branching = alpha / beta
    penalty = PENALTY * max(branching - 0.999, 0.0) ** 2
    loglik = log_term - integral_term - penalty
    out = np.float32(-loglik)
    if _want_trace:
        return out, res
    return out


# revision 6
# speedup vs baseline: 1.2241x; 1.2241x over previous
"""Exponential Hawkes process negative log-likelihood on 8 Trainium2 cores.

Math (reference):
    R_0 = 0;  R_i = exp(-beta*(t_i - t_{i-1})) * (1 + R_{i-1})
    lam_i = mu + alpha * R_i
    nll = -[ sum_i log(lam_i) - mu*T - (alpha/beta) * sum_i (1 - exp(-beta*(T - t_i)))
             - 1000 * relu(alpha/beta - 0.999)^2 ]

Strategy (blocked scan, per the sharding hint):
  - The kernel's device input is dt_i = t_i - t_{i-1} (the same event data,
    differenced on the host during sharding and shipped as bf16 -- half the
    HBM traffic of f32 t, and it removes the shifted-subtract from the
    device's Vector engine, whose serial scan is the bottleneck).
  - Shard the 8.4M event axis across 8 cores, each shard prefixed with a
    1536-event halo so the incoming recurrence carry is reproduced locally
    (exp(-beta * halo_span) underflows to 0 in f32; verified host-side).
    Core 0 is front-padded with dt = 1e6, whose a = exp(-beta dt) = 0
    resets the recurrence exactly.
  - Per core the (halo+shard) dt sequence is laid out [128, C]: partition p
    owns a contiguous chunk of C events.  Per column tile:
      a    = exp(-beta*dt)                 (ACT, bf16 in / f32 out)
      B    = scan: B_c = a_c*(1+B_{c-1})   (DVE tensor_tensor_scan,
                                            ~2 cyc/elem, chained via the
                                            previous tile's last column)
      logl = Ln(alpha*B + mu), accumulated (ACT accum_out) -- valid for
             columns >= W_c because the cross-partition carry correction
             decays to exactly 0 in f32 there; W_c is verified host-side.
  - Input loads are issued on the Sync engine's DMA ring; the (small)
    outputs go out on the Scalar engine's ring so they are not queued
    behind the streaming input loads (in-order queues: head-of-line
    blocking otherwise adds ~5-10us to every output).
  - The cross-partition carry (128 values/core) and the first W_c columns'
    corrected log terms are finished on the host in f64 from B_head/B_end.
  - The integral sum_i exp(-beta*(T - t_i)) is computed entirely on the
    host in f64: only events within ~700/beta of T contribute above
    1e-300, a tiny tail (for the reference data ~300 events).
  - Per-(partition, tile) log partial sums come back; the host masks the
    halo entries and reduces everything in f64.
"""

import numpy as np
import ml_dtypes

# Problem constants (hardcoded per task instructions).
N = 8_388_608          # total events
M = 8                  # cores
S = N // M             # events per shard (1,048,576)
H = 1536               # halo events prepended to each shard; equals a
                       # tile-boundary prefix so the halo split is aligned
L = S + H              # per-core sequence length
P = 128                # SBUF partitions
C = L // P             # columns per partition (8204)
EPS = 1e-8
PENALTY = 1000.0
PAD_GAP = 1.0e6        # core-0 pad dt; exp(-beta*PAD_GAP) == 0 in f32

# Column tiles (start, width).  Small head tiles so the serial scan chain
# starts while the DMA queues are still ramping; moderate tail tile so the
# last Ln finishes quickly.  Halo boundary (H = 1536) falls after tile 1.
# Fallback config with a 1536-wide head tile for smaller beta (the carry
# window W_c must fit inside tile 0).
_TILES_A = [(0, 512), (512, 1024), (1536, 2216), (3752, 2696), (6448, 1756)]
_TILES_B = [(0, 1536), (1536, 2048), (3584, 2296), (5880, 2324)]
assert sum(w for _, w in _TILES_A) == C and sum(w for _, w in _TILES_B) == C
assert _TILES_A[0][1] + _TILES_A[1][1] == H and _TILES_B[0][1] == H

_PROGRAM_CACHE: dict = {}


def _softplus64(x: float) -> float:
    return float(np.logaddexp(0.0, np.float64(x)))


def _build_program(beta: float, mu: float, alpha: float,
                   tiles: tuple, w_carry: int):
    import concourse.bacc as bacc
    import concourse.mybir as mybir
    from concourse.tile import TileContext

    f32 = mybir.dt.float32
    bf16 = mybir.dt.bfloat16
    AF = mybir.ActivationFunctionType
    OP = mybir.AluOpType
    Wc = w_carry
    NT = len(tiles)
    assert 0 < Wc <= tiles[0][1]
    FMAX = max(w for _, w in tiles)

    # This kernel interleaves Exp and Ln activations per tile.  The stock
    # table chooser picks the first act-func-set containing each function,
    # which alternates between an Exp-only and an Ln-only set and inserts an
    # ACT_TABLE_LOAD (~1.3us) at every switch.  Hide Exp/Ln from all sets
    # except the combined one (order/indices preserved) so both functions
    # resolve to a single resident table.
    if not getattr(bacc, "_hawkes_act_tables_patched", False):
        _orig_get_tables = bacc.get_activation_tables

        def _patched_get_tables(module_arch):
            tabs = _orig_get_tables(module_arch)
            both = {name for name, s in tabs.items()
                    if AF.Exp in s and AF.Ln in s}
            if both:
                keep = next(iter(sorted(both)))
                tabs = {
                    name: (s if name == keep
                           else s - {AF.Exp, AF.Ln})
                    for name, s in tabs.items()
                }
            return tabs

        bacc.get_activation_tables = _patched_get_tables
        bacc._hawkes_act_tables_patched = True

    nc = bacc.Bacc()
    dt_in = nc.dram_tensor("dt", [P, C], bf16, kind="ExternalInput")
    out_stats = nc.dram_tensor("out_stats", [P, NT + 1], f32,
                               kind="ExternalOutput")
    out_bhead = nc.dram_tensor("out_bhead", [P, Wc], f32,
                               kind="ExternalOutput")

    with TileContext(nc) as tc:
        with tc.tile_pool(name="pers", bufs=1) as pers, \
             tc.tile_pool(name="work", bufs=3) as work:
            stats = pers.tile([P, NT + 1], f32)
            musb = pers.tile([P, 1], f32)
            nc.gpsimd.memset(stats[:], 0.0)
            nc.gpsimd.memset(musb[:], float(mu))

            prev_b = None
            for j, (c0, w) in enumerate(tiles):
                dtt = work.tile([P, FMAX], bf16, tag="dt")
                nc.sync.dma_start(dtt[:, :w], dt_in[:, c0:c0 + w])

                at = work.tile([P, FMAX], f32, tag="a")
                nc.scalar.activation(at[:, :w], dtt[:, :w], AF.Exp,
                                     scale=float(-beta))

                bt = work.tile([P, FMAX], f32, tag="b")
                init = 0.0 if j == 0 else prev_b
                nc.vector.tensor_tensor_scan(
                    bt[:, :w], at[:, :w], at[:, :w], init,
                    op0=OP.mult, op1=OP.add)

                # log-lik over carry-free columns (B == R there, exactly);
                # Ln output values are discarded, only accum_out matters.
                lnl = work.tile([P, FMAX], bf16, tag="lnl")
                if j == 0:
                    # ship the carry-head block as soon as it exists (on
                    # the Scalar DMA ring: the Sync ring is busy streaming
                    # the input and would head-of-line block it)
                    nc.scalar.dma_start(out_bhead[:], bt[:, :Wc])
                    nc.scalar.activation(lnl[:, :w - Wc], bt[:, Wc:w],
                                         AF.Ln, scale=float(alpha),
                                         bias=musb[:],
                                         accum_out=stats[:, 0:1])
                else:
                    nc.scalar.activation(lnl[:, :w], bt[:, :w],
                                         AF.Ln, scale=float(alpha),
                                         bias=musb[:],
                                         accum_out=stats[:, j:j + 1])
                if j == NT - 1:
                    nc.vector.tensor_copy(stats[:, NT:NT + 1],
                                          bt[:, w - 1:w])
                prev_b = bt[:, w - 1:w]

            nc.scalar.dma_start(out_stats[:], stats[:], single_packet=True)

    nc.finalize()
    return nc


def _get_program(beta, mu, alpha, tiles, w_carry):
    key = (repr(beta), repr(mu), repr(alpha), tuple(tiles), w_carry)
    prog = _PROGRAM_CACHE.get(key)
    if prog is None:
        prog = _build_program(beta, mu, alpha, tiles, w_carry)
        _PROGRAM_CACHE[key] = prog
    return prog


def kernel(event_times, raw_mu, raw_alpha, raw_beta, _want_trace=False):
    from concourse.bass_utils import run_bass_kernel_spmd

    ev_full = np.ascontiguousarray(np.asarray(event_times, dtype=np.float32))
    assert ev_full.shape == (N,), ev_full.shape
    mu = _softplus64(float(np.asarray(raw_mu))) + EPS
    alpha = _softplus64(float(np.asarray(raw_alpha))) + EPS
    beta = _softplus64(float(np.asarray(raw_beta))) + EPS
    T = float(ev_full[-1])

    # dt in f32 (same subtraction the reference's recurrence sees), then
    # bf16 for the device; dt[0] has no predecessor -> a must be 0.
    dt_full = np.empty(N, np.float32)
    dt_full[0] = PAD_GAP
    np.subtract(ev_full[1:], ev_full[:-1], out=dt_full[1:])
    dt16_full = dt_full.astype(ml_dtypes.bfloat16)

    # halo sufficiency: the carry truncated at each shard/halo start must
    # have decayed to 0 (in f32) before the first real event.
    halo_span = ev_full[np.arange(1, M) * S] - ev_full[np.arange(1, M) * S - H]
    if not np.all(beta * halo_span.astype(np.float64) > 120.0):
        raise RuntimeError(f"halo H={H} insufficient for beta={beta}")

    # Per-core inputs and host-side fixup metadata
    in_maps = []
    prevs = []     # predecessor event time of each partition chunk
    t2ds = []      # per-core [P, C] event-time windows (for f64 fixups)
    wc_req = 0
    for k in range(M):
        if k == 0:
            win_dt = np.empty(L, ml_dtypes.bfloat16)
            win_dt[:H] = ml_dtypes.bfloat16(PAD_GAP)
            win_dt[H:] = dt16_full[:S]
            win_t = np.empty(L, np.float32)
            win_t[:H] = ev_full[0] - np.float32(PAD_GAP)
            win_t[H:] = ev_full[:S]
        else:
            win_dt = dt16_full[k * S - H:(k + 1) * S]
            win_t = ev_full[k * S - H:(k + 1) * S]
        pv = np.empty(P, np.float64)
        pv[0] = np.float64(win_t[0]) - 1.0
        pv[1:] = win_t[C - 1:L - 1:C]
        t2d = win_t.reshape(P, C)
        # first column where beta*(t - t_chunk0) > 110 (margin over the
        # f32 exp underflow at ~104); beyond it the carry/init correction
        # has decayed to exactly 0 in f32
        past = t2d > (t2d[:, 0:1] + np.float32(110.0 / beta))
        if k == 0:
            past = past[1:]   # pad row: a = 0 resets exactly, never binds
        if not past[:, -1].all():
            wc_req = C        # pathological: no underflow within the row
        else:
            wc_req = max(wc_req, int(past.argmax(axis=1).max()))
        in_maps.append({"dt": np.ascontiguousarray(win_dt.reshape(P, C))})
        prevs.append(pv)
        t2ds.append(t2d)

    tiles = _TILES_A
    w_carry = min(-(-max(wc_req + 64, 128) // 64) * 64, tiles[0][1])
    if wc_req + 16 > w_carry:
        tiles = _TILES_B
        w_carry = min(-(-max(wc_req + 64, 128) // 64) * 64, tiles[0][1])
        if wc_req + 16 > w_carry:
            raise RuntimeError(
                f"carry window {wc_req} exceeds tile width {tiles[0][1]}; "
                f"beta={beta} too small for this build")

    prog = _get_program(beta, mu, alpha, tuple(tiles), w_carry)
    res = run_bass_kernel_spmd(prog, in_maps, list(range(M)),
                               trace=_want_trace)

    NT = len(tiles)
    Wc = w_carry
    log_term = np.float64(0.0)
    for k in range(M):
        r = res.results[k]
        st = r["out_stats"].astype(np.float64)          # [P, NT+1]
        lg = st[:, 0:NT]
        for j, (c0, w) in enumerate(tiles):
            if c0 + w <= H:      # partition-0 columns of this tile = halo
                lg[0, j] = 0.0
        log_term += lg.sum()

        # Host-side carry, all f64.  The device scanned each partition chunk
        # with init 0; the true carry-in K[p] adds P_c*K[p] with
        # P_c = prod_{j<=c} a_j ~= exp(-beta*(t_c - prev_p)), which has
        # decayed to exactly 0 (f32) for c >= Wc.
        t2d = t2ds[k].astype(np.float64)
        pv = prevs[k]
        bend = st[:, NT]                                         # [P]
        Pend = np.exp(-beta * (t2d[:, C - 1] - pv))              # [P]
        K = np.empty(P, np.float64)
        rend = 0.0
        for p in range(P):
            K[p] = rend
            rend = bend[p] + Pend[p] * rend
        bhead = r["out_bhead"].astype(np.float64)                # [P, Wc]
        Phead = np.exp(-beta * (t2d[:, :Wc] - pv[:, None]))      # [P, Wc]
        R = bhead + Phead * K[:, None]
        lncorr = np.log(mu + alpha * R)                          # [P, Wc]
        log_term += lncorr[1:, :].sum()                          # row 0: halo

    # Integral term fully on host (f64): only events within ~700/beta of T
    # contribute above 1e-300.
    lo = int(np.searchsorted(ev_full, np.float32(T - 700.0 / beta)))
    int_exp = float(np.exp(-beta * (np.float64(T) -
                                    ev_full[lo:].astype(np.float64))).sum())
    integral_term = mu * T + (alpha / beta) * (N - int_exp)

    branching = alpha / beta
    penalty = PENALTY * max(branching - 0.999, 0.0) ** 2
    loglik = log_term - integral_term - penalty
    out = np.float32(-loglik)
    if _want_trace:
        return out, res
    return out


# revision 9
# speedup vs baseline: 1.2962x; 1.0589x over previous
"""Exponential Hawkes process negative log-likelihood on 8 Trainium2 cores.

Math (reference):
    R_0 = 0;  R_i = exp(-beta*(t_i - t_{i-1})) * (1 + R_{i-1})
    lam_i = mu + alpha * R_i
    nll = -[ sum_i log(lam_i) - mu*T - (alpha/beta) * sum_i (1 - exp(-beta*(T - t_i)))
             - 1000 * relu(alpha/beta - 0.999)^2 ]

Strategy (blocked scan, per the sharding hint):
  - The kernel's device input is dt_i = t_i - t_{i-1} (the same event data,
    differenced on the host during sharding and shipped as bf16 -- half the
    HBM traffic of f32 t, and it removes the shifted-subtract from the
    device's Vector engine, whose serial scan is the bottleneck).
  - Shard the 8.4M event axis across 8 cores, each shard prefixed with a
    1536-event halo so the incoming recurrence carry is reproduced locally
    (exp(-beta * halo_span) underflows to 0 in f32; verified host-side).
    Core 0 is front-padded with dt = 1e6, whose a = exp(-beta dt) = 0
    resets the recurrence exactly.
  - Per core the (halo+shard) dt sequence is laid out [128, C]: partition p
    owns a contiguous chunk of C events.  Per column tile:
      a    = exp(-beta*dt)                 (ACT, bf16 in / f32 out)
      B    = scan: B_c = a_c*(1+B_{c-1})   (DVE tensor_tensor_scan,
                                            ~2 cyc/elem, chained via the
                                            previous tile's last column)
      logl = Ln(alpha*B + mu), accumulated (ACT accum_out) -- valid for
             columns >= W_c because the cross-partition carry correction
             decays to exactly 0 in f32 there; W_c is verified host-side.
  - Input loads are issued on the Sync engine's DMA ring; the (small)
    outputs go out on the Scalar engine's ring so they are not queued
    behind the streaming input loads (in-order queues: head-of-line
    blocking otherwise adds ~5-10us to every output).
  - The cross-partition carry (128 values/core) and the first W_c columns'
    corrected log terms are finished on the host in f64 from B_head/B_end.
  - The integral sum_i exp(-beta*(T - t_i)) is computed entirely on the
    host in f64: only events within ~700/beta of T contribute above
    1e-300, a tiny tail (for the reference data ~300 events).
  - Per-(partition, tile) log partial sums come back; the host masks the
    halo entries and reduces everything in f64.
"""

import numpy as np
import ml_dtypes

# Problem constants (hardcoded per task instructions).
N = 8_388_608          # total events
M = 8                  # cores
S = N // M             # events per shard (1,048,576)
H = 1024               # halo events prepended to each shard; equals a
                       # tile-boundary prefix so the halo split is aligned
L = S + H              # per-core sequence length
P = 128                # SBUF partitions
C = L // P             # columns per partition (8204)
EPS = 1e-8
PENALTY = 1000.0
PAD_GAP = 1.0e6        # core-0 pad dt; exp(-beta*PAD_GAP) == 0 in f32

# Column tiles (start, width).  Small head tiles so the serial scan chain
# starts while the DMA queues are still ramping; moderate tail tile so the
# last Ln finishes quickly.  Halo boundary (H = 1536) falls after tile 1.
# Fallback config with a 1536-wide head tile for smaller beta (the carry
# window W_c must fit inside tile 0).
_TILES_A = [(0, 320), (320, 704), (1024, 1216), (2240, 1984), (4224, 2432),
            (6656, 1544)]
_TILES_B = [(0, 1024), (1024, 2048), (3072, 2432), (5504, 2696)]
assert sum(w for _, w in _TILES_A) == C and sum(w for _, w in _TILES_B) == C
assert _TILES_A[0][1] + _TILES_A[1][1] == H and _TILES_B[0][1] == H

_PROGRAM_CACHE: dict = {}


def _softplus64(x: float) -> float:
    return float(np.logaddexp(0.0, np.float64(x)))


def _build_program(beta: float, mu: float, alpha: float,
                   tiles: tuple, w_carry: int):
    import concourse.bacc as bacc
    import concourse.mybir as mybir
    from concourse.tile import TileContext

    f32 = mybir.dt.float32
    bf16 = mybir.dt.bfloat16
    AF = mybir.ActivationFunctionType
    OP = mybir.AluOpType
    Wc = w_carry
    NT = len(tiles)
    assert 0 < Wc <= tiles[0][1]
    FMAX = max(w for _, w in tiles)

    # This kernel interleaves Exp and Ln activations per tile.  The stock
    # table chooser picks the first act-func-set containing each function,
    # which alternates between an Exp-only and an Ln-only set and inserts an
    # ACT_TABLE_LOAD (~1.3us) at every switch.  Hide Exp/Ln from all sets
    # except the combined one (order/indices preserved) so both functions
    # resolve to a single resident table.
    if not getattr(bacc, "_hawkes_act_tables_patched", False):
        _orig_get_tables = bacc.get_activation_tables

        def _patched_get_tables(module_arch):
            tabs = _orig_get_tables(module_arch)
            both = {name for name, s in tabs.items()
                    if AF.Exp in s and AF.Ln in s}
            if both:
                keep = next(iter(sorted(both)))
                tabs = {
                    name: (s if name == keep
                           else s - {AF.Exp, AF.Ln})
                    for name, s in tabs.items()
                }
            return tabs

        bacc.get_activation_tables = _patched_get_tables
        bacc._hawkes_act_tables_patched = True

    nc = bacc.Bacc()
    dt_in = nc.dram_tensor("dt", [P, C], bf16, kind="ExternalInput")
    out_stats = nc.dram_tensor("out_stats", [P, NT + 1], f32,
                               kind="ExternalOutput")
    out_bhead = nc.dram_tensor("out_bhead", [P, Wc], f32,
                               kind="ExternalOutput")

    with TileContext(nc) as tc:
        with tc.tile_pool(name="pers", bufs=1) as pers, \
             tc.tile_pool(name="work", bufs=3) as work:
            stats = pers.tile([P, NT + 1], f32)
            musb = pers.tile([P, 1], f32)
            nc.gpsimd.memset(stats[:], 0.0)
            nc.gpsimd.memset(musb[:], float(mu))

            prev_b = None
            for j, (c0, w) in enumerate(tiles):
                dtt = work.tile([P, FMAX], bf16, tag="dt")
                nc.sync.dma_start(dtt[:, :w], dt_in[:, c0:c0 + w])

                at = work.tile([P, FMAX], f32, tag="a")
                nc.scalar.activation(at[:, :w], dtt[:, :w], AF.Exp,
                                     scale=float(-beta))

                bt = work.tile([P, FMAX], f32, tag="b")
                init = 0.0 if j == 0 else prev_b
                nc.vector.tensor_tensor_scan(
                    bt[:, :w], at[:, :w], at[:, :w], init,
                    op0=OP.mult, op1=OP.add)

                # log-lik over carry-free columns (B == R there, exactly);
                # Ln output values are discarded, only accum_out matters.
                lnl = work.tile([P, FMAX], bf16, tag="lnl")
                if j == 0:
                    # ship the carry-head block as soon as it exists (on
                    # the Scalar DMA ring: the Sync ring is busy streaming
                    # the input and would head-of-line block it)
                    nc.scalar.dma_start(out_bhead[:], bt[:, :Wc])
                    if w > Wc:
                        nc.scalar.activation(lnl[:, :w - Wc], bt[:, Wc:w],
                                             AF.Ln, scale=float(alpha),
                                             bias=musb[:],
                                             accum_out=stats[:, 0:1])
                else:
                    nc.scalar.activation(lnl[:, :w], bt[:, :w],
                                         AF.Ln, scale=float(alpha),
                                         bias=musb[:],
                                         accum_out=stats[:, j:j + 1])
                if j == NT - 1:
                    nc.vector.tensor_copy(stats[:, NT:NT + 1],
                                          bt[:, w - 1:w])
                prev_b = bt[:, w - 1:w]

            nc.scalar.dma_start(out_stats[:], stats[:], single_packet=True)

    nc.finalize()
    return nc


def _get_program(beta, mu, alpha, tiles, w_carry):
    key = (repr(beta), repr(mu), repr(alpha), tuple(tiles), w_carry)
    prog = _PROGRAM_CACHE.get(key)
    if prog is None:
        prog = _build_program(beta, mu, alpha, tiles, w_carry)
        _PROGRAM_CACHE[key] = prog
    return prog


def kernel(event_times, raw_mu, raw_alpha, raw_beta, _want_trace=False):
    from concourse.bass_utils import run_bass_kernel_spmd

    ev_full = np.ascontiguousarray(np.asarray(event_times, dtype=np.float32))
    assert ev_full.shape == (N,), ev_full.shape
    mu = _softplus64(float(np.asarray(raw_mu))) + EPS
    alpha = _softplus64(float(np.asarray(raw_alpha))) + EPS
    beta = _softplus64(float(np.asarray(raw_beta))) + EPS
    T = float(ev_full[-1])

    # dt in f32 (same subtraction the reference's recurrence sees), then
    # bf16 for the device; dt[0] has no predecessor -> a must be 0.
    dt_full = np.empty(N, np.float32)
    dt_full[0] = PAD_GAP
    np.subtract(ev_full[1:], ev_full[:-1], out=dt_full[1:])
    dt16_full = dt_full.astype(ml_dtypes.bfloat16)

    # halo sufficiency: the carry truncated at each shard/halo start must
    # have decayed to 0 (in f32) before the first real event.
    halo_span = ev_full[np.arange(1, M) * S] - ev_full[np.arange(1, M) * S - H]
    if not np.all(beta * halo_span.astype(np.float64) > 120.0):
        raise RuntimeError(f"halo H={H} insufficient for beta={beta}")

    # Per-core inputs and host-side fixup metadata
    in_maps = []
    prevs = []     # predecessor event time of each partition chunk
    t2ds = []      # per-core [P, C] event-time windows (for f64 fixups)
    wc_req = 0
    for k in range(M):
        if k == 0:
            win_dt = np.empty(L, ml_dtypes.bfloat16)
            win_dt[:H] = ml_dtypes.bfloat16(PAD_GAP)
            win_dt[H:] = dt16_full[:S]
            win_t = np.empty(L, np.float32)
            win_t[:H] = ev_full[0] - np.float32(PAD_GAP)
            win_t[H:] = ev_full[:S]
        else:
            win_dt = dt16_full[k * S - H:(k + 1) * S]
            win_t = ev_full[k * S - H:(k + 1) * S]
        pv = np.empty(P, np.float64)
        pv[0] = np.float64(win_t[0]) - 1.0
        pv[1:] = win_t[C - 1:L - 1:C]
        t2d = win_t.reshape(P, C)
        # first column where beta*(t - t_chunk0) > 110 (margin over the
        # f32 exp underflow at ~104); beyond it the carry/init correction
        # has decayed to exactly 0 in f32
        past = t2d > (t2d[:, 0:1] + np.float32(110.0 / beta))
        if k == 0:
            past = past[1:]   # pad row: a = 0 resets exactly, never binds
        if not past[:, -1].all():
            wc_req = C        # pathological: no underflow within the row
        else:
            wc_req = max(wc_req, int(past.argmax(axis=1).max()))
        in_maps.append({"dt": np.ascontiguousarray(win_dt.reshape(P, C))})
        prevs.append(pv)
        t2ds.append(t2d)

    tiles = _TILES_A
    w_carry = min(-(-max(wc_req + 64, 128) // 64) * 64, tiles[0][1])
    if wc_req + 16 > w_carry:
        tiles = _TILES_B
        w_carry = min(-(-max(wc_req + 64, 128) // 64) * 64, tiles[0][1])
        if wc_req + 16 > w_carry:
            raise RuntimeError(
                f"carry window {wc_req} exceeds tile width {tiles[0][1]}; "
                f"beta={beta} too small for this build")

    prog = _get_program(beta, mu, alpha, tuple(tiles), w_carry)
    res = run_bass_kernel_spmd(prog, in_maps, list(range(M)),
                               trace=_want_trace)

    NT = len(tiles)
    Wc = w_carry
    log_term = np.float64(0.0)
    for k in range(M):
        r = res.results[k]
        st = r["out_stats"].astype(np.float64)          # [P, NT+1]
        lg = st[:, 0:NT]
        for j, (c0, w) in enumerate(tiles):
            if c0 + w <= H:      # partition-0 columns of this tile = halo
                lg[0, j] = 0.0
        log_term += lg.sum()

        # Host-side carry, all f64.  The device scanned each partition chunk
        # with init 0; the true carry-in K[p] adds P_c*K[p] with
        # P_c = prod_{j<=c} a_j ~= exp(-beta*(t_c - prev_p)), which has
        # decayed to exactly 0 (f32) for c >= Wc.
        t2d = t2ds[k].astype(np.float64)
        pv = prevs[k]
        bend = st[:, NT]                                         # [P]
        Pend = np.exp(-beta * (t2d[:, C - 1] - pv))              # [P]
        K = np.empty(P, np.float64)
        rend = 0.0
        for p in range(P):
            K[p] = rend
            rend = bend[p] + Pend[p] * rend
        bhead = r["out_bhead"].astype(np.float64)                # [P, Wc]
        Phead = np.exp(-beta * (t2d[:, :Wc] - pv[:, None]))      # [P, Wc]
        R = bhead + Phead * K[:, None]
        lncorr = np.log(mu + alpha * R)                          # [P, Wc]
        log_term += lncorr[1:, :].sum()                          # row 0: halo

    # Integral term fully on host (f64): only events within ~700/beta of T
    # contribute above 1e-300.
    lo = int(np.searchsorted(ev_full, np.float32(T - 700.0 / beta)))
    int_exp = float(np.exp(-beta * (np.float64(T) -
                                    ev_full[lo:].astype(np.float64))).sum())
    integral_term = mu * T + (alpha / beta) * (N - int_exp)

    branching = alpha / beta
    penalty = PENALTY * max(branching - 0.999, 0.0) ** 2
    loglik = log_term - integral_term - penalty
    out = np.float32(-loglik)
    if _want_trace:
        return out, res
    return out
